# revision 26
# baseline (speedup 1.0000x reference)
"""GCN layer (GCNConv + ReLU) Bass kernel for 8 Trainium2 NeuronCores.

Reference computation (PyG GCNConv with self-loops, eval mode):
    deg  = in-degree(dst) + 1                       (self loops included)
    norm_e = deg^-1/2[src_e] * deg^-1/2[dst_e]
    out  = relu( segment_sum_dst( (x @ W)[src] * norm ) + b )
         = relu( segment_sum_dst( x[src] * norm ) @ W + b )   (W applied last)

Device strategy (per core, SPMD over 8 cores):
  - dst nodes are bin-packed into uniform "chunks" of <=256 slot positions;
    each core owns cpc chunks.  Edges are grouped by dst chunk and split
    into two streams by src range (int16 gather-index limit).
  - Each core's chunks are rank-ordered by edge count (descending) so the
    c-th chunk has a similar block count on every core; per-rank loop
    bounds are the max over cores, keeping the SPMD program shared while
    nearly eliminating pad gathers.
  - For each chunk: dma_gather (bf16 rows, 4 SWDGE queues round-robin)
    pulls x[src] rows from HBM into SBUF, a one-hot selection matrix
    S[e, dst_slot] = (iota==dst_e)*norm_e is built on VectorE (or on
    ScalarE via abs/relu, to offload the DVE<->GpSimd shared SBUF port),
    and TensorE accumulates
        aggT[cin, dst_slot] += sum_e G[e, cin] * S[e, dst_slot]
    into PSUM across all edge blocks of the chunk.
  - Finally aggT @ W is computed (W stationary), bias+ReLU applied on
    ScalarE, and the transposed [cout, dst] tile is stored; the host
    unpermutes/transposes to the full [N, C] output.
"""

import os

import numpy as np

import concourse.bacc as bacc
import concourse.bass as bass
import concourse.mybir as mybir
import concourse.tile as tile
from concourse.bass_utils import run_bass_kernel_spmd

N_CORES = 8
CHUNK_W = 256  # dst slots per chunk == PSUM tile free dim
SPLIT = 32768  # gather table split point (int16 index limit)
NODES_PER_CHUNK = 250

# knobs (env overridable for experiments)
N_QUEUES = int(os.environ.get("GCN_QUEUES", "4"))  # SWDGE queues (1..4)
ACT_MOD = int(os.environ.get("GCN_ACT_MOD", "3"))  # every ACT_MOD-th sel on ScalarE
SEL_STREAM = os.environ.get("GCN_SEL_STREAM", "1") == "1"  # host-built sel via DMA
ACT_FRAC = float(os.environ.get("GCN_ACT_FRAC", "0.0"))  # tail sel blocks on ScalarE
SEL_FP8 = os.environ.get("GCN_SEL_FP8", "1") == "1"  # 0/1 sel in fp8, dinv folded out
SELF_STATIC = os.environ.get("GCN_SELF_STATIC", "1") == "1"  # self loops via static DMA
GATHER_DTYPE = os.environ.get("GCN_GATHER_DTYPE", "bf16")
GMAX = int(os.environ.get("GCN_GMAX", "8"))  # blocks (128 idx) per dma_gather
SINGLE_PACKET = os.environ.get("GCN_SP", "1") == "1"

LAST_RUN_INFO = {}


def _host_prep(x, edge_index):
    """Host-side sharding: chunk assignment, edge bucketing, index layout."""
    N, C = x.shape
    src = np.asarray(edge_index[0], dtype=np.int64)
    dst = np.asarray(edge_index[1], dtype=np.int64)
    loops = np.arange(N, dtype=np.int64)
    src = np.concatenate([src, loops])
    dst = np.concatenate([dst, loops])

    deg = np.bincount(dst, minlength=N)
    dinv = (1.0 / np.sqrt(deg.astype(np.float64))).astype(np.float32)
    norm = dinv[src] * dinv[dst]

    cpc = int(np.ceil(N / (N_CORES * NODES_PER_CHUNK)))
    nchunks = N_CORES * cpc

    self_static = SELF_STATIC and SEL_STREAM and SEL_FP8
    if self_static:
        # Contiguous chunks: chunk c owns nodes [Q*c, Q*(c+1)) so the
        # appended self-loop rows are a contiguous x slice per chunk,
        # loaded by static DMA instead of dma_gather descriptors.
        # Per-rank loop bounds absorb the lost degree balancing.
        Q = NODES_PER_CHUNK
        chunk_of = np.arange(N, dtype=np.int64) // Q
        slot_of = np.arange(N, dtype=np.int64) % Q
        # drop the appended loops (the last N entries) from the edge streams
        src = src[:-N]
        dst = dst[:-N]
        norm_stream = norm[:-N]
    else:
        # Balance chunks by in-degree: snake round-robin over degree-sorted
        # nodes.
        order = np.argsort(-deg, kind="stable")
        r = np.arange(N)
        pos = r % nchunks
        rnd = r // nchunks
        ch = np.where(rnd % 2 == 0, pos, nchunks - 1 - pos)
        chunk_of = np.empty(N, np.int64)
        slot_of = np.empty(N, np.int64)
        chunk_of[order] = ch
        slot_of[order] = rnd
        norm_stream = norm
    assert slot_of.max() < CHUNK_W

    e_chunk = chunk_of[dst]
    e_slot = slot_of[dst]
    islo = src < SPLIT
    # Dedup repeated srcs within a (chunk, stream) group: one gathered row can
    # fan out to several dst slots via multiple nonzeros in its sel row.
    dedup = SEL_STREAM and ACT_FRAC == 0
    key = e_chunk * 2 + (~islo).astype(np.int64)
    if dedup:
        kk = (key << 32) | src
        uk, inv_u = np.unique(kk, return_inverse=True)
        ug = (uk >> 32).astype(np.int64)
        usrc = (uk & 0xFFFFFFFF).astype(np.int64)
        ucnt = np.bincount(ug, minlength=2 * nchunks)
        cnt_lo = ucnt[0::2]
        cnt_hi = ucnt[1::2]
    else:
        cnt_lo = np.bincount(e_chunk[islo], minlength=nchunks)
        cnt_hi = np.bincount(e_chunk[~islo], minlength=nchunks)

    # Rank-order each core's chunks by total count (desc) and take per-rank
    # maxima across cores so all cores share one set of loop bounds.
    rank_of_chunk = np.empty(nchunks, np.int64)
    chunk_at = np.empty((N_CORES, cpc), np.int64)
    for k in range(N_CORES):
        mine = np.arange(k * cpc, (k + 1) * cpc)
        o = np.argsort(-(cnt_lo[mine] + cnt_hi[mine]), kind="stable")
        chunk_at[k] = mine[o]
        rank_of_chunk[mine[o]] = np.arange(cpc)

    B_lo = [
        int(np.ceil(max(cnt_lo[chunk_at[k][c]] for k in range(N_CORES)) / 128))
        for c in range(cpc)
    ]
    B_hi = [
        int(np.ceil(max(cnt_hi[chunk_at[k][c]] for k in range(N_CORES)) / 128))
        for c in range(cpc)
    ]
    B_lo = [max(b, 1) for b in B_lo]
    B_hi = [max(b, 1) for b in B_hi]
    B_self_pre = 2 if (SELF_STATIC and SEL_STREAM and SEL_FP8) else 0
    B_tot = sum(B_lo) + sum(B_hi) + cpc * B_self_pre
    # block offset of each rank's lo / hi / self regions in the flat layout
    off_lo, off_hi, off_self = [], [], []
    acc = 0
    for c in range(cpc):
        off_lo.append(acc)
        acc += B_lo[c]
        off_hi.append(acc)
        acc += B_hi[c]
        off_self.append(acc)
        acc += B_self_pre
    assert acc == B_tot

    # row positions in the flat per-core layout
    e_core = e_chunk // cpc
    off_lo_a = np.array(off_lo)
    off_hi_a = np.array(off_hi)
    S = B_tot * 128
    flat_idx = np.zeros((N_CORES, S), np.int64)
    flat_dst = np.zeros((N_CORES, S), np.float32)
    flat_nrm = np.zeros((N_CORES, S), np.float32)
    if dedup:
        gstart_u = np.zeros(2 * nchunks, np.int64)
        gstart_u[1:] = np.cumsum(ucnt)[:-1]
        rank_u = np.arange(len(uk)) - gstart_u[ug]
        u_chunk = ug // 2
        u_islo = ug % 2 == 0
        u_rankc = rank_of_chunk[u_chunk]
        u_core = u_chunk // cpc
        blk_u = np.where(u_islo, off_lo_a[u_rankc], off_hi_a[u_rankc])
        pos_u = blk_u * 128 + rank_u
        flat_idx[u_core, pos_u] = np.where(u_islo, usrc, usrc - SPLIT)
        pos_e = pos_u[inv_u]  # per edge: its gathered row's position
    else:
        e_rank = rank_of_chunk[e_chunk]
        perm = np.argsort(key, kind="stable")
        ks = key[perm]
        gsz = np.bincount(key, minlength=2 * nchunks)
        gstart = np.zeros(2 * nchunks, np.int64)
        gstart[1:] = np.cumsum(gsz)[:-1]
        rank_in_g = np.arange(len(ks)) - gstart[ks]

        sp = src[perm]
        ep_slot = e_slot[perm]
        ep_nrm = norm_stream[perm]
        ep_core = e_core[perm]
        ep_rank = e_rank[perm]
        ep_islo = ks % 2 == 0
        blk_base = np.where(ep_islo, off_lo_a[ep_rank], off_hi_a[ep_rank])
        pos = blk_base * 128 + rank_in_g
        flat_idx[ep_core, pos] = np.where(ep_islo, sp, sp - SPLIT)
        flat_dst[ep_core, pos] = ep_slot.astype(np.float32)
        flat_nrm[ep_core, pos] = ep_nrm
    assert flat_idx.max() < SPLIT and flat_idx.min() >= 0
    flat_idx16 = flat_idx.astype(np.int16)

    sel_stream = None
    use_fp8 = SEL_FP8
    if SEL_STREAM:
        import ml_dtypes

        # Dense one-hot selection matrices, host-built: block gb row p holds
        # the edge weight at column (dst slot) of the edge at flat position
        # gb*128+p.  In fp8 mode the weight is exactly 1.0 (0/1 one-hot is
        # exact in fp8) and both dinv factors are folded outside: dinv[src]
        # into the gathered x rows, dinv[dst] into a host-side post-scale
        # (valid because bias == 0 so relu commutes with the positive scale).
        sdt = ml_dtypes.float8_e4m3 if use_fp8 else ml_dtypes.bfloat16
        sel_stream = np.zeros((N_CORES, 128, B_tot * CHUNK_W), sdt)
        if dedup:
            vals = np.ones(len(e_slot)) if use_fp8 else norm_stream
            lin = (e_core * 128 + pos_e % 128) * (B_tot * CHUNK_W) + (
                pos_e // 128
            ) * CHUNK_W + e_slot
            uu, invu = np.unique(lin, return_inverse=True)
            sums = np.bincount(invu, weights=vals.astype(np.float64))
            sel_stream.reshape(-1)[uu] = sums.astype(sdt)
        else:
            vals = np.ones_like(ep_nrm) if use_fp8 else ep_nrm
            if use_fp8:
                vals = vals * (ep_nrm != 0)
            sel_stream[
                ep_core, pos % 128, (pos // 128) * CHUNK_W + ep_slot
            ] = vals.astype(sdt)
        if self_static:
            # identity one-hots for the two self blocks of every rank: the
            # node at slot s sits at (partition s%128, self block s//128)
            for c in range(cpc):
                for b in range(2):
                    lo_s = b * 128
                    hi_s = min(NODES_PER_CHUNK, (b + 1) * 128)
                    p = np.arange(hi_s - lo_s)
                    colbase = (off_self[c] + b) * CHUNK_W
                    sel_stream[:, p, colbase + lo_s + p] = 1.0

    per_core = []
    for k in range(N_CORES):
        # gather idx layout: logical idx i -> [i%16 (replicated x8), i//16]
        v = flat_idx16[k].reshape(B_tot * 8, 16).T
        per_core.append(
            dict(
                idx=np.ascontiguousarray(np.tile(v, (8, 1))),
                dstslot=np.ascontiguousarray(flat_dst[k].reshape(B_tot, 128).T),
                normv=np.ascontiguousarray(flat_nrm[k].reshape(B_tot, 128).T),
                sel=sel_stream[k] if SEL_STREAM else None,
            )
        )

    B_act = [
        min(int(round(ACT_FRAC * (B_lo[c] + B_hi[c]))), B_hi[c]) if SEL_STREAM else 0
        for c in range(cpc)
    ]
    B_self = 2 if self_static else 0

    meta = dict(
        N=N,
        C=C,
        dinv=dinv,
        cpc=cpc,
        nchunks=nchunks,
        B_tot=B_tot,
        B_lo=B_lo,
        B_hi=B_hi,
        B_act=B_act,
        B_self=B_self,
        off_lo=off_lo,
        off_hi=off_hi,
        off_self=off_self,
        self_static=self_static,
        chunk_at=chunk_at,
        chunk_of=chunk_of,
        slot_of=slot_of,
        rank_of_chunk=rank_of_chunk,
    )
    return per_core, meta


def _pieces(nblocks, max_blocks):
    """Split nblocks into near-even contiguous pieces of <= max_blocks."""
    npieces = -(-nblocks // max_blocks)
    out = []
    a = 0
    for p in range(npieces):
        b = a + (nblocks - a) // (npieces - p) + (1 if (nblocks - a) % (npieces - p) else 0)
        out.append((a, b))
        a = b
    return out


def _build_program(meta):
    f32 = mybir.dt.float32
    bf16 = mybir.dt.bfloat16
    i16 = mybir.dt.int16
    gdt = bf16 if GATHER_DTYPE == "bf16" else f32
    N, C, cpc = meta["N"], meta["C"], meta["cpc"]
    B_lo, B_hi = meta["B_lo"], meta["B_hi"]
    off_lo, off_hi = meta["off_lo"], meta["off_hi"]
    B_tot = meta["B_tot"]
    BLmax, BHmax = max(B_lo), max(B_hi)

    nc = bacc.Bacc(
        None, target_bir_lowering=False, debug=False, num_swdge_queues=N_QUEUES
    )

    x_d = nc.dram_tensor("xg", [N, C], gdt, kind="ExternalInput")
    idx_d = nc.dram_tensor("idx", [128, B_tot * 8], i16, kind="ExternalInput")
    sdt = mybir.dt.float8e4 if SEL_FP8 else bf16
    need_tables = (not SEL_STREAM) or any(meta["B_act"])
    if SEL_STREAM:
        sel_d = nc.dram_tensor(
            "sel", [128, B_tot * CHUNK_W], sdt, kind="ExternalInput"
        )
    if need_tables:
        dst_d = nc.dram_tensor("dstslot", [128, B_tot], f32, kind="ExternalInput")
        nrm_d = nc.dram_tensor("normv", [128, B_tot], f32, kind="ExternalInput")
        ndst_d = nc.dram_tensor("ndstslot", [128, B_tot], f32, kind="ExternalInput")
        nnrm_d = nc.dram_tensor("nnormv", [128, B_tot], f32, kind="ExternalInput")
        iota_d = nc.dram_tensor("iota", [128, CHUNK_W], f32, kind="ExternalInput")
    B_act = meta["B_act"]
    B_self = meta["B_self"]
    self_static = meta["self_static"]
    if self_static:
        xself_d = nc.dram_tensor(
            "xself", [cpc * 2 * 128, C], gdt, kind="ExternalInput"
        )
    Bmax = max(bl + bh for bl, bh in zip(B_lo, B_hi)) + B_self
    w_d = nc.dram_tensor("weight", [C, C], f32, kind="ExternalInput")
    b_d = nc.dram_tensor("bias", [128, 1], f32, kind="ExternalInput")
    out_d = nc.dram_tensor("out", [128, cpc * CHUNK_W], bf16, kind="ExternalOutput")

    ie = mybir.AluOpType.is_equal
    mu = mybir.AluOpType.mult
    AB = mybir.ActivationFunctionType.Abs
    RL = mybir.ActivationFunctionType.Relu

    with tile.TileContext(nc) as tc:
        with (
            tc.tile_pool(name="const", bufs=1) as constp,
            tc.tile_pool(name="gat_lo", bufs=4) as glop,
            tc.tile_pool(name="gat_hi", bufs=4) as ghip,
            tc.tile_pool(name="slf", bufs=3) as slfp,
            tc.tile_pool(name="sel", bufs=(4 if SEL_STREAM else 8)) as selp,
            tc.tile_pool(name="tmp", bufs=4) as tmpp,
            tc.tile_pool(name="aggs", bufs=3) as aggsp,
            tc.tile_pool(name="outs", bufs=3) as outsp,
            tc.tile_pool(name="pagg", bufs=4, space="PSUM") as pagg,
            tc.tile_pool(name="pout", bufs=2, space="PSUM") as pout,
        ):
            w_t = constp.tile([C, C], f32, tag="w")
            nc.sync.dma_start(w_t[:], w_d[:])
            bias_t = constp.tile([128, 1], f32, tag="bias")
            nc.sync.dma_start(bias_t[:], b_d[:])
            # stage the idx table load so early chunks' gathers start ASAP
            idx_t = constp.tile([128, B_tot * 8], i16, tag="idx")
            cuts = [0]
            for cb in (1, 3, 8):
                if cb < cpc:
                    cuts.append(off_lo[cb] * 8)
            cuts.append(B_tot * 8)
            for a, b in zip(cuts[:-1], cuts[1:]):
                if b > a:
                    nc.sync.dma_start(idx_t[:, a:b], idx_d[:, a:b])
            if need_tables:
                iota_t = constp.tile([128, CHUNK_W], f32, tag="iota")
                nc.sync.dma_start(iota_t[:], iota_d[:])
                dst_t = constp.tile([128, B_tot], f32, tag="dst")
                nc.sync.dma_start(dst_t[:], dst_d[:])
                nrm_t = constp.tile([128, B_tot], f32, tag="nrm")
                nc.sync.dma_start(nrm_t[:], nrm_d[:])
                ndst_t = constp.tile([128, B_tot], f32, tag="ndst")
                nc.sync.dma_start(ndst_t[:], ndst_d[:])
                nnrm_t = constp.tile([128, B_tot], f32, tag="nnrm")
                nc.sync.dma_start(nnrm_t[:], nnrm_d[:])

            x_lo = x_d[0:SPLIT, :]
            x_hi = x_d[SPLIT:N, :]

            qc = 0  # rotate dma_gather instructions across SWDGE queues
            sc = 0  # sel build counter (for DVE/ACT split)
            for c in range(cpc):
                bl, bh = B_lo[c], B_hi[c]
                ol, oh = off_lo[c], off_hi[c]
                # dma_gather with single_packet caps at 64 descs/engine = 1024
                # idxs per instruction (HW-verified: 1280 wedges the device).
                glo_t = glop.tile([128, BLmax, C], gdt, tag="glo")
                for kb0, kb1 in _pieces(bl, GMAX):
                    n = (kb1 - kb0) * 128
                    nc.gpsimd.dma_gather(
                        glo_t[:, kb0:kb1, :],
                        x_lo,
                        idx_t[:, (ol + kb0) * 8 : (ol + kb1) * 8],
                        n,
                        n,
                        C,
                        queue_num=qc % N_QUEUES,
                        single_packet=SINGLE_PACKET,
                    )
                    qc += 1
                ghi_t = ghip.tile([128, BHmax, C], gdt, tag="ghi")
                for kb0, kb1 in _pieces(bh, GMAX):
                    n = (kb1 - kb0) * 128
                    nc.gpsimd.dma_gather(
                        ghi_t[:, kb0:kb1, :],
                        x_hi,
                        idx_t[:, (oh + kb0) * 8 : (oh + kb1) * 8],
                        n,
                        n,
                        C,
                        queue_num=qc % N_QUEUES,
                        single_packet=SINGLE_PACKET,
                    )
                    qc += 1

                if self_static:
                    self_t = slfp.tile([128, 2, C], gdt, tag="slf")
                    nc.sync.dma_start(
                        self_t[:],
                        xself_d[c * 256 : (c + 1) * 256, :].rearrange(
                            "(b p) c -> p b c", p=128
                        ),
                    )
                agg_t = pagg.tile([128, CHUNK_W], mybir.dt.float32, tag="agg")
                B = bl + bh + B_self
                n_act = B_act[c]
                n_stream = B - n_act
                if SEL_STREAM:
                    selc_t = selp.tile([128, Bmax * CHUNK_W], sdt, tag="selc")
                    nc.sync.dma_start(
                        selc_t[:, : n_stream * CHUNK_W],
                        sel_d[:, ol * CHUNK_W : (ol + n_stream) * CHUNK_W],
                    )
                for b in range(B):
                    gb = (ol + b) if b < bl else (oh + b - bl)
                    if SEL_STREAM and b < n_stream:
                        sel_ap = selc_t[:, b * CHUNK_W : (b + 1) * CHUNK_W]
                    elif SEL_STREAM:
                        # tail blocks: build on ScalarE (own SBUF ports; no
                        # contention with the GpSimd gather ucode)
                        sel_t = selp.tile([128, CHUNK_W], bf16, tag="sela")
                        tmp_t = tmpp.tile([128, CHUNK_W], bf16, tag="tmp")
                        nc.scalar.activation(
                            tmp_t[:], iota_t[:], AB, bias=ndst_t[:, gb : gb + 1]
                        )
                        nc.scalar.activation(
                            sel_t[:],
                            tmp_t[:],
                            RL,
                            bias=nrm_t[:, gb : gb + 1],
                            scale=nnrm_t[:, gb : gb + 1],
                        )
                        sel_ap = sel_t[:]
                    else:
                        sel_t = selp.tile([128, CHUNK_W], gdt, tag="sel")
                        if sc % ACT_MOD == ACT_MOD - 1:
                            # ScalarE: t = |iota - dst|; sel = relu(nrm - nrm*t)
                            tmp_t = tmpp.tile([128, CHUNK_W], bf16, tag="tmp")
                            nc.scalar.activation(
                                tmp_t[:], iota_t[:], AB, bias=ndst_t[:, gb : gb + 1]
                            )
                            nc.scalar.activation(
                                sel_t[:],
                                tmp_t[:],
                                RL,
                                bias=nrm_t[:, gb : gb + 1],
                                scale=nnrm_t[:, gb : gb + 1],
                            )
                        else:
                            nc.vector.tensor_scalar(
                                sel_t[:],
                                iota_t[:],
                                dst_t[:, gb : gb + 1],
                                nrm_t[:, gb : gb + 1],
                                ie,
                                mu,
                            )
                        sel_ap = sel_t[:]
                    sc += 1
                    if b < bl:
                        g_ap = glo_t[:, b, :]
                    elif b < bl + bh:
                        g_ap = ghi_t[:, b - bl, :]
                    else:
                        g_ap = self_t[:, b - bl - bh, :]
                    nc.tensor.matmul(
                        agg_t[:],
                        lhsT=g_ap,
                        rhs=sel_ap,
                        start=(b == 0),
                        stop=(b == B - 1),
                    )
                aggs_t = aggsp.tile([128, CHUNK_W], f32, tag="aggs")
                nc.scalar.copy(aggs_t[:], agg_t[:])
                outp_t = pout.tile([128, CHUNK_W], mybir.dt.float32, tag="outp")
                nc.tensor.matmul(
                    outp_t[:], lhsT=w_t[:], rhs=aggs_t[:], start=True, stop=True
                )
                outs_t = outsp.tile([128, CHUNK_W], bf16, tag="outs")
                nc.scalar.activation(
                    outs_t[:],
                    outp_t[:],
                    RL,
                    bias=bias_t[:, 0:1],
                    scale=1.0,
                )
                nc.sync.dma_start(out_d[:, c * CHUNK_W : (c + 1) * CHUNK_W], outs_t[:])
    nc.compile()
    return nc


def _make_in_maps(x, weight, bias, per_core, meta):
    import ml_dtypes


    gnp = ml_dtypes.bfloat16 if GATHER_DTYPE == "bf16" else np.float32
    xf = np.asarray(x, dtype=np.float32)
    if SEL_STREAM and SEL_FP8:
        xf = xf * meta["dinv"][:, None]
    xg = np.ascontiguousarray(xf.astype(gnp))
    xselfs = None
    if meta["self_static"]:
        cpc = meta["cpc"]
        Q = NODES_PER_CHUNK
        xselfs = []
        for k in range(N_CORES):
            xs = np.zeros((cpc * 2 * 128, xg.shape[1]), gnp)
            for c in range(cpc):
                q = int(meta["chunk_at"][k][c])
                xs[c * 256 : c * 256 + Q] = xg[q * Q : (q + 1) * Q]
            xselfs.append(xs)
    iota = np.tile(np.arange(CHUNK_W, dtype=np.float32), (128, 1))
    w = np.ascontiguousarray(np.asarray(weight, dtype=np.float32))
    bvec = np.zeros((128, 1), np.float32)
    bvec[: len(bias), 0] = np.asarray(bias, dtype=np.float32)
    in_maps = []
    for k in range(N_CORES):
        pc = per_core[k]
        m = dict(xg=xg, idx=pc["idx"], weight=w, bias=bvec)
        if (not SEL_STREAM) or any(meta["B_act"]):
            m.update(
                dstslot=pc["dstslot"],
                normv=pc["normv"],
                ndstslot=-pc["dstslot"],
                nnormv=-pc["normv"],
                iota=iota,
            )
        if SEL_STREAM:
            m["sel"] = pc["sel"]
        if meta["self_static"]:
            m["xself"] = xselfs[k]
        in_maps.append(m)
    return in_maps


def _unshard(results, meta):
    cpc = meta["cpc"]
    outs = [np.asarray(results[k]["out"], dtype=np.float32) for k in range(N_CORES)]
    big = np.concatenate(outs, axis=1)  # [128, ncores*cpc*CHUNK_W]
    chunk_of = meta["chunk_of"]
    rank = meta["rank_of_chunk"][chunk_of]
    core = chunk_of // cpc
    col = core * (cpc * CHUNK_W) + rank * CHUNK_W + meta["slot_of"]
    out = np.ascontiguousarray(big[:, col].T)
    if SEL_STREAM and SEL_FP8:
        # relu(dinv_dst * z) == dinv_dst * relu(z) since dinv_dst > 0
        out *= meta["dinv"][:, None]
    return out


def kernel(x, edge_index, weight, bias):
    global SEL_FP8
    x = np.asarray(x)
    if SEL_FP8 and np.any(np.asarray(bias) != 0):
        SEL_FP8 = False  # fp8 one-hot path relies on bias==0
    per_core, meta = _host_prep(x, edge_index)
    nc = _build_program(meta)
    in_maps = _make_in_maps(x, np.asarray(weight), np.asarray(bias), per_core, meta)
    res = run_bass_kernel_spmd(
        nc,
        in_maps,
        list(range(N_CORES)),
        trace=os.environ.get("GCN_TRACE", "0") == "1",
    )
    LAST_RUN_INFO["exec_time_ns"] = res.exec_time_ns
    LAST_RUN_INFO["meta"] = {
        k: v for k, v in meta.items() if np.isscalar(v)
    } | dict(B_tot=meta["B_tot"])
    return _unshard(res.results, meta)


# revision 27
# speedup vs baseline: 1.0195x; 1.0195x over previous
"""GCN layer (GCNConv + ReLU) Bass kernel for 8 Trainium2 NeuronCores.

Reference computation (PyG GCNConv with self-loops, eval mode):
    deg  = in-degree(dst) + 1                       (self loops included)
    norm_e = deg^-1/2[src_e] * deg^-1/2[dst_e]
    out  = relu( segment_sum_dst( (x @ W)[src] * norm ) + b )
         = relu( segment_sum_dst( x[src] * norm ) @ W + b )   (W applied last)

Device strategy (per core, SPMD over 8 cores):
  - dst nodes are bin-packed into uniform "chunks" of <=256 slot positions;
    each core owns cpc chunks.  Edges are grouped by dst chunk and split
    into two streams by src range (int16 gather-index limit).
  - Each core's chunks are rank-ordered by edge count (descending) so the
    c-th chunk has a similar block count on every core; per-rank loop
    bounds are the max over cores, keeping the SPMD program shared while
    nearly eliminating pad gathers.
  - For each chunk: dma_gather (bf16 rows, 4 SWDGE queues round-robin)
    pulls x[src] rows from HBM into SBUF, a one-hot selection matrix
    S[e, dst_slot] = (iota==dst_e)*norm_e is built on VectorE (or on
    ScalarE via abs/relu, to offload the DVE<->GpSimd shared SBUF port),
    and TensorE accumulates
        aggT[cin, dst_slot] += sum_e G[e, cin] * S[e, dst_slot]
    into PSUM across all edge blocks of the chunk.
  - Finally aggT @ W is computed (W stationary), bias+ReLU applied on
    ScalarE, and the transposed [cout, dst] tile is stored; the host
    unpermutes/transposes to the full [N, C] output.
"""

import os

import numpy as np

import concourse.bacc as bacc
import concourse.bass as bass
import concourse.mybir as mybir
import concourse.tile as tile
from concourse.bass_utils import run_bass_kernel_spmd

N_CORES = 8
CHUNK_W = 256  # dst slots per chunk == PSUM tile free dim
SPLIT = 32768  # gather table split point (int16 index limit)
NODES_PER_CHUNK = 250

# knobs (env overridable for experiments)
N_QUEUES = int(os.environ.get("GCN_QUEUES", "4"))  # SWDGE queues (1..4)
ACT_MOD = int(os.environ.get("GCN_ACT_MOD", "3"))  # every ACT_MOD-th sel on ScalarE
SEL_STREAM = os.environ.get("GCN_SEL_STREAM", "1") == "1"  # host-built sel via DMA
ACT_FRAC = float(os.environ.get("GCN_ACT_FRAC", "0.0"))  # tail sel blocks on ScalarE
SEL_FP8 = os.environ.get("GCN_SEL_FP8", "1") == "1"  # 0/1 sel in fp8, dinv folded out
SELF_STATIC = os.environ.get("GCN_SELF_STATIC", "1") == "1"  # self loops via static DMA
GATHER_DTYPE = os.environ.get("GCN_GATHER_DTYPE", "bf16")
GMAX = int(os.environ.get("GCN_GMAX", "8"))  # blocks (128 idx) per dma_gather
SINGLE_PACKET = os.environ.get("GCN_SP", "1") == "1"

LAST_RUN_INFO = {}


def _host_prep(x, edge_index):
    """Host-side sharding: chunk assignment, edge bucketing, index layout."""
    N, C = x.shape
    src = np.asarray(edge_index[0], dtype=np.int64)
    dst = np.asarray(edge_index[1], dtype=np.int64)
    loops = np.arange(N, dtype=np.int64)
    src = np.concatenate([src, loops])
    dst = np.concatenate([dst, loops])

    deg = np.bincount(dst, minlength=N)
    dinv = (1.0 / np.sqrt(deg.astype(np.float64))).astype(np.float32)
    norm = dinv[src] * dinv[dst]

    cpc = int(np.ceil(N / (N_CORES * NODES_PER_CHUNK)))
    nchunks = N_CORES * cpc

    self_static = SELF_STATIC and SEL_STREAM and SEL_FP8
    if self_static:
        # Contiguous chunks: chunk c owns nodes [Q*c, Q*(c+1)) so the
        # appended self-loop rows are a contiguous x slice per chunk,
        # loaded by static DMA instead of dma_gather descriptors.
        # Per-rank loop bounds absorb the lost degree balancing.
        Q = NODES_PER_CHUNK
        chunk_of = np.arange(N, dtype=np.int64) // Q
        slot_of = np.arange(N, dtype=np.int64) % Q
        # drop the appended loops (the last N entries) from the edge streams
        src = src[:-N]
        dst = dst[:-N]
        norm_stream = norm[:-N]
    else:
        # Balance chunks by in-degree: snake round-robin over degree-sorted
        # nodes.
        order = np.argsort(-deg, kind="stable")
        r = np.arange(N)
        pos = r % nchunks
        rnd = r // nchunks
        ch = np.where(rnd % 2 == 0, pos, nchunks - 1 - pos)
        chunk_of = np.empty(N, np.int64)
        slot_of = np.empty(N, np.int64)
        chunk_of[order] = ch
        slot_of[order] = rnd
        norm_stream = norm
    assert slot_of.max() < CHUNK_W

    e_chunk = chunk_of[dst]
    e_slot = slot_of[dst]
    islo = src < SPLIT
    # Dedup repeated srcs within a (chunk, stream) group: one gathered row can
    # fan out to several dst slots via multiple nonzeros in its sel row.
    dedup = SEL_STREAM and ACT_FRAC == 0
    key = e_chunk * 2 + (~islo).astype(np.int64)
    if dedup:
        kk = (key << 32) | src
        uk, inv_u = np.unique(kk, return_inverse=True)
        ug = (uk >> 32).astype(np.int64)
        usrc = (uk & 0xFFFFFFFF).astype(np.int64)
        ucnt = np.bincount(ug, minlength=2 * nchunks)
        cnt_lo = ucnt[0::2]
        cnt_hi = ucnt[1::2]
    else:
        cnt_lo = np.bincount(e_chunk[islo], minlength=nchunks)
        cnt_hi = np.bincount(e_chunk[~islo], minlength=nchunks)

    # Rank-order each core's chunks by total count (desc) and take per-rank
    # maxima across cores so all cores share one set of loop bounds.
    rank_of_chunk = np.empty(nchunks, np.int64)
    chunk_at = np.empty((N_CORES, cpc), np.int64)
    for k in range(N_CORES):
        mine = np.arange(k * cpc, (k + 1) * cpc)
        o = np.argsort(-(cnt_lo[mine] + cnt_hi[mine]), kind="stable")
        chunk_at[k] = mine[o]
        rank_of_chunk[mine[o]] = np.arange(cpc)

    B_lo = [
        int(np.ceil(max(cnt_lo[chunk_at[k][c]] for k in range(N_CORES)) / 128))
        for c in range(cpc)
    ]
    B_hi = [
        int(np.ceil(max(cnt_hi[chunk_at[k][c]] for k in range(N_CORES)) / 128))
        for c in range(cpc)
    ]
    B_lo = [max(b, 1) for b in B_lo]
    B_hi = [max(b, 1) for b in B_hi]
    B_self_pre = 2 if (SELF_STATIC and SEL_STREAM and SEL_FP8) else 0
    B_tot = sum(B_lo) + sum(B_hi) + cpc * B_self_pre
    # block offset of each rank's lo / hi / self regions in the flat layout
    off_lo, off_hi, off_self = [], [], []
    acc = 0
    for c in range(cpc):
        off_lo.append(acc)
        acc += B_lo[c]
        off_hi.append(acc)
        acc += B_hi[c]
        off_self.append(acc)
        acc += B_self_pre
    assert acc == B_tot

    # row positions in the flat per-core layout
    e_core = e_chunk // cpc
    off_lo_a = np.array(off_lo)
    off_hi_a = np.array(off_hi)
    S = B_tot * 128
    flat_idx = np.zeros((N_CORES, S), np.int64)
    flat_dst = np.zeros((N_CORES, S), np.float32)
    flat_nrm = np.zeros((N_CORES, S), np.float32)
    if dedup:
        gstart_u = np.zeros(2 * nchunks, np.int64)
        gstart_u[1:] = np.cumsum(ucnt)[:-1]
        rank_u = np.arange(len(uk)) - gstart_u[ug]
        u_chunk = ug // 2
        u_islo = ug % 2 == 0
        u_rankc = rank_of_chunk[u_chunk]
        u_core = u_chunk // cpc
        blk_u = np.where(u_islo, off_lo_a[u_rankc], off_hi_a[u_rankc])
        pos_u = blk_u * 128 + rank_u
        flat_idx[u_core, pos_u] = np.where(u_islo, usrc, usrc - SPLIT)
        pos_e = pos_u[inv_u]  # per edge: its gathered row's position
    else:
        e_rank = rank_of_chunk[e_chunk]
        perm = np.argsort(key, kind="stable")
        ks = key[perm]
        gsz = np.bincount(key, minlength=2 * nchunks)
        gstart = np.zeros(2 * nchunks, np.int64)
        gstart[1:] = np.cumsum(gsz)[:-1]
        rank_in_g = np.arange(len(ks)) - gstart[ks]

        sp = src[perm]
        ep_slot = e_slot[perm]
        ep_nrm = norm_stream[perm]
        ep_core = e_core[perm]
        ep_rank = e_rank[perm]
        ep_islo = ks % 2 == 0
        blk_base = np.where(ep_islo, off_lo_a[ep_rank], off_hi_a[ep_rank])
        pos = blk_base * 128 + rank_in_g
        flat_idx[ep_core, pos] = np.where(ep_islo, sp, sp - SPLIT)
        flat_dst[ep_core, pos] = ep_slot.astype(np.float32)
        flat_nrm[ep_core, pos] = ep_nrm
    assert flat_idx.max() < SPLIT and flat_idx.min() >= 0
    flat_idx16 = flat_idx.astype(np.int16)

    sel_stream = None
    use_fp8 = SEL_FP8
    if SEL_STREAM:
        import ml_dtypes

        # Dense one-hot selection matrices, host-built: block gb row p holds
        # the edge weight at column (dst slot) of the edge at flat position
        # gb*128+p.  In fp8 mode the weight is exactly 1.0 (0/1 one-hot is
        # exact in fp8) and both dinv factors are folded outside: dinv[src]
        # into the gathered x rows, dinv[dst] into a host-side post-scale
        # (valid because bias == 0 so relu commutes with the positive scale).
        sdt = ml_dtypes.float8_e4m3 if use_fp8 else ml_dtypes.bfloat16
        sel_stream = np.zeros((N_CORES, 128, B_tot * CHUNK_W), sdt)
        if dedup:
            vals = np.ones(len(e_slot)) if use_fp8 else norm_stream
            lin = (e_core * 128 + pos_e % 128) * (B_tot * CHUNK_W) + (
                pos_e // 128
            ) * CHUNK_W + e_slot
            uu, invu = np.unique(lin, return_inverse=True)
            sums = np.bincount(invu, weights=vals.astype(np.float64))
            sel_stream.reshape(-1)[uu] = sums.astype(sdt)
        else:
            vals = np.ones_like(ep_nrm) if use_fp8 else ep_nrm
            if use_fp8:
                vals = vals * (ep_nrm != 0)
            sel_stream[
                ep_core, pos % 128, (pos // 128) * CHUNK_W + ep_slot
            ] = vals.astype(sdt)
        if self_static:
            # identity one-hots for the two self blocks of every rank: the
            # node at slot s sits at (partition s%128, self block s//128)
            for c in range(cpc):
                for b in range(2):
                    lo_s = b * 128
                    hi_s = min(NODES_PER_CHUNK, (b + 1) * 128)
                    p = np.arange(hi_s - lo_s)
                    colbase = (off_self[c] + b) * CHUNK_W
                    sel_stream[:, p, colbase + lo_s + p] = 1.0

    per_core = []
    for k in range(N_CORES):
        # gather idx layout: logical idx i -> [i%16 (replicated x8), i//16]
        v = flat_idx16[k].reshape(B_tot * 8, 16).T
        per_core.append(
            dict(
                idx=np.ascontiguousarray(np.tile(v, (8, 1))),
                dstslot=np.ascontiguousarray(flat_dst[k].reshape(B_tot, 128).T),
                normv=np.ascontiguousarray(flat_nrm[k].reshape(B_tot, 128).T),
                sel=sel_stream[k] if SEL_STREAM else None,
            )
        )

    B_act = [
        min(int(round(ACT_FRAC * (B_lo[c] + B_hi[c]))), B_hi[c]) if SEL_STREAM else 0
        for c in range(cpc)
    ]
    B_self = 2 if self_static else 0

    meta = dict(
        N=N,
        C=C,
        dinv=dinv,
        cpc=cpc,
        nchunks=nchunks,
        B_tot=B_tot,
        B_lo=B_lo,
        B_hi=B_hi,
        B_act=B_act,
        B_self=B_self,
        off_lo=off_lo,
        off_hi=off_hi,
        off_self=off_self,
        self_static=self_static,
        chunk_at=chunk_at,
        chunk_of=chunk_of,
        slot_of=slot_of,
        rank_of_chunk=rank_of_chunk,
    )
    return per_core, meta


def _pieces(nblocks, max_blocks):
    """Split nblocks into near-even contiguous pieces of <= max_blocks."""
    npieces = -(-nblocks // max_blocks)
    out = []
    a = 0
    for p in range(npieces):
        b = a + (nblocks - a) // (npieces - p) + (1 if (nblocks - a) % (npieces - p) else 0)
        out.append((a, b))
        a = b
    return out


def _build_program(meta):
    f32 = mybir.dt.float32
    bf16 = mybir.dt.bfloat16
    i16 = mybir.dt.int16
    gdt = bf16 if GATHER_DTYPE == "bf16" else f32
    N, C, cpc = meta["N"], meta["C"], meta["cpc"]
    B_lo, B_hi = meta["B_lo"], meta["B_hi"]
    off_lo, off_hi = meta["off_lo"], meta["off_hi"]
    B_tot = meta["B_tot"]
    BLmax, BHmax = max(B_lo), max(B_hi)

    nc = bacc.Bacc(
        None, target_bir_lowering=False, debug=False, num_swdge_queues=N_QUEUES
    )

    x_d = nc.dram_tensor("xg", [N, C], gdt, kind="ExternalInput")
    idx_d = nc.dram_tensor("idx", [128, B_tot * 8], i16, kind="ExternalInput")
    sdt = mybir.dt.float8e4 if SEL_FP8 else bf16
    need_tables = (not SEL_STREAM) or any(meta["B_act"])
    if SEL_STREAM:
        sel_d = nc.dram_tensor(
            "sel", [128, B_tot * CHUNK_W], sdt, kind="ExternalInput"
        )
    if need_tables:
        dst_d = nc.dram_tensor("dstslot", [128, B_tot], f32, kind="ExternalInput")
        nrm_d = nc.dram_tensor("normv", [128, B_tot], f32, kind="ExternalInput")
        ndst_d = nc.dram_tensor("ndstslot", [128, B_tot], f32, kind="ExternalInput")
        nnrm_d = nc.dram_tensor("nnormv", [128, B_tot], f32, kind="ExternalInput")
        iota_d = nc.dram_tensor("iota", [128, CHUNK_W], f32, kind="ExternalInput")
    B_act = meta["B_act"]
    B_self = meta["B_self"]
    self_static = meta["self_static"]
    if self_static:
        xself_d = nc.dram_tensor(
            "xself", [cpc * 2 * 128, C], gdt, kind="ExternalInput"
        )
    Bmax = max(bl + bh for bl, bh in zip(B_lo, B_hi)) + B_self
    w_d = nc.dram_tensor("weight", [C, C], f32, kind="ExternalInput")
    b_d = nc.dram_tensor("bias", [128, 1], f32, kind="ExternalInput")
    out_d = nc.dram_tensor("out", [128, cpc * CHUNK_W], bf16, kind="ExternalOutput")

    ie = mybir.AluOpType.is_equal
    mu = mybir.AluOpType.mult
    AB = mybir.ActivationFunctionType.Abs
    RL = mybir.ActivationFunctionType.Relu

    with tile.TileContext(nc) as tc:
        with (
            tc.tile_pool(name="const", bufs=1) as constp,
            tc.tile_pool(name="gat_lo", bufs=4) as glop,
            tc.tile_pool(name="gat_hi", bufs=4) as ghip,
            tc.tile_pool(name="slf", bufs=3) as slfp,
            tc.tile_pool(name="sel", bufs=(4 if SEL_STREAM else 8)) as selp,
            tc.tile_pool(name="tmp", bufs=4) as tmpp,
            tc.tile_pool(name="aggs", bufs=4) as aggsp,
            tc.tile_pool(name="outs", bufs=4) as outsp,
            tc.tile_pool(name="pagg", bufs=6, space="PSUM") as pagg,
            tc.tile_pool(name="pout", bufs=2, space="PSUM") as pout,
        ):
            w_t = constp.tile([C, C], f32, tag="w")
            nc.sync.dma_start(w_t[:], w_d[:])
            bias_t = constp.tile([128, 1], f32, tag="bias")
            nc.sync.dma_start(bias_t[:], b_d[:])
            # stage the idx table load so early chunks' gathers start ASAP
            idx_t = constp.tile([128, B_tot * 8], i16, tag="idx")
            cuts = [0]
            for cb in (1, 3, 8):
                if cb < cpc:
                    cuts.append(off_lo[cb] * 8)
            cuts.append(B_tot * 8)
            for a, b in zip(cuts[:-1], cuts[1:]):
                if b > a:
                    nc.sync.dma_start(idx_t[:, a:b], idx_d[:, a:b])
            if need_tables:
                iota_t = constp.tile([128, CHUNK_W], f32, tag="iota")
                nc.sync.dma_start(iota_t[:], iota_d[:])
                dst_t = constp.tile([128, B_tot], f32, tag="dst")
                nc.sync.dma_start(dst_t[:], dst_d[:])
                nrm_t = constp.tile([128, B_tot], f32, tag="nrm")
                nc.sync.dma_start(nrm_t[:], nrm_d[:])
                ndst_t = constp.tile([128, B_tot], f32, tag="ndst")
                nc.sync.dma_start(ndst_t[:], ndst_d[:])
                nnrm_t = constp.tile([128, B_tot], f32, tag="nnrm")
                nc.sync.dma_start(nnrm_t[:], nnrm_d[:])

            x_lo = x_d[0:SPLIT, :]
            x_hi = x_d[SPLIT:N, :]

            qc = 0  # rotate dma_gather instructions across SWDGE queues
            sc = 0  # sel build counter (for DVE/ACT split)
            for c in range(cpc):
                bl, bh = B_lo[c], B_hi[c]
                ol, oh = off_lo[c], off_hi[c]
                # dma_gather with single_packet caps at 64 descs/engine = 1024
                # idxs per instruction (HW-verified: 1280 wedges the device).
                glo_t = glop.tile([128, BLmax, C], gdt, tag="glo")
                for kb0, kb1 in _pieces(bl, GMAX):
                    n = (kb1 - kb0) * 128
                    nc.gpsimd.dma_gather(
                        glo_t[:, kb0:kb1, :],
                        x_lo,
                        idx_t[:, (ol + kb0) * 8 : (ol + kb1) * 8],
                        n,
                        n,
                        C,
                        queue_num=qc % N_QUEUES,
                        single_packet=SINGLE_PACKET,
                    )
                    qc += 1
                ghi_t = ghip.tile([128, BHmax, C], gdt, tag="ghi")
                for kb0, kb1 in _pieces(bh, GMAX):
                    n = (kb1 - kb0) * 128
                    nc.gpsimd.dma_gather(
                        ghi_t[:, kb0:kb1, :],
                        x_hi,
                        idx_t[:, (oh + kb0) * 8 : (oh + kb1) * 8],
                        n,
                        n,
                        C,
                        queue_num=qc % N_QUEUES,
                        single_packet=SINGLE_PACKET,
                    )
                    qc += 1

                if self_static:
                    self_t = slfp.tile([128, 2, C], gdt, tag="slf")
                    nc.sync.dma_start(
                        self_t[:],
                        xself_d[c * 256 : (c + 1) * 256, :].rearrange(
                            "(b p) c -> p b c", p=128
                        ),
                    )
                agg_t = pagg.tile([128, CHUNK_W], mybir.dt.float32, tag="agg")
                B = bl + bh + B_self
                n_act = B_act[c]
                n_stream = B - n_act
                if SEL_STREAM:
                    selc_t = selp.tile([128, Bmax * CHUNK_W], sdt, tag="selc")
                    nc.sync.dma_start(
                        selc_t[:, : n_stream * CHUNK_W],
                        sel_d[:, ol * CHUNK_W : (ol + n_stream) * CHUNK_W],
                    )
                for b in range(B):
                    gb = (ol + b) if b < bl else (oh + b - bl)
                    if SEL_STREAM and b < n_stream:
                        sel_ap = selc_t[:, b * CHUNK_W : (b + 1) * CHUNK_W]
                    elif SEL_STREAM:
                        # tail blocks: build on ScalarE (own SBUF ports; no
                        # contention with the GpSimd gather ucode)
                        sel_t = selp.tile([128, CHUNK_W], bf16, tag="sela")
                        tmp_t = tmpp.tile([128, CHUNK_W], bf16, tag="tmp")
                        nc.scalar.activation(
                            tmp_t[:], iota_t[:], AB, bias=ndst_t[:, gb : gb + 1]
                        )
                        nc.scalar.activation(
                            sel_t[:],
                            tmp_t[:],
                            RL,
                            bias=nrm_t[:, gb : gb + 1],
                            scale=nnrm_t[:, gb : gb + 1],
                        )
                        sel_ap = sel_t[:]
                    else:
                        sel_t = selp.tile([128, CHUNK_W], gdt, tag="sel")
                        if sc % ACT_MOD == ACT_MOD - 1:
                            # ScalarE: t = |iota - dst|; sel = relu(nrm - nrm*t)
                            tmp_t = tmpp.tile([128, CHUNK_W], bf16, tag="tmp")
                            nc.scalar.activation(
                                tmp_t[:], iota_t[:], AB, bias=ndst_t[:, gb : gb + 1]
                            )
                            nc.scalar.activation(
                                sel_t[:],
                                tmp_t[:],
                                RL,
                                bias=nrm_t[:, gb : gb + 1],
                                scale=nnrm_t[:, gb : gb + 1],
                            )
                        else:
                            nc.vector.tensor_scalar(
                                sel_t[:],
                                iota_t[:],
                                dst_t[:, gb : gb + 1],
                                nrm_t[:, gb : gb + 1],
                                ie,
                                mu,
                            )
                        sel_ap = sel_t[:]
                    sc += 1
                    if b < bl:
                        g_ap = glo_t[:, b, :]
                    elif b < bl + bh:
                        g_ap = ghi_t[:, b - bl, :]
                    else:
                        g_ap = self_t[:, b - bl - bh, :]
                    nc.tensor.matmul(
                        agg_t[:],
                        lhsT=g_ap,
                        rhs=sel_ap,
                        start=(b == 0),
                        stop=(b == B - 1),
                    )
                aggs_t = aggsp.tile([128, CHUNK_W], f32, tag="aggs")
                nc.scalar.copy(aggs_t[:], agg_t[:])
                outp_t = pout.tile([128, CHUNK_W], mybir.dt.float32, tag="outp")
                nc.tensor.matmul(
                    outp_t[:], lhsT=w_t[:], rhs=aggs_t[:], start=True, stop=True
                )
                outs_t = outsp.tile([128, CHUNK_W], bf16, tag="outs")
                nc.scalar.activation(
                    outs_t[:],
                    outp_t[:],
                    RL,
                    bias=bias_t[:, 0:1],
                    scale=1.0,
                )
                nc.sync.dma_start(out_d[:, c * CHUNK_W : (c + 1) * CHUNK_W], outs_t[:])
    nc.compile()
    return nc


def _make_in_maps(x, weight, bias, per_core, meta):
    import ml_dtypes


    gnp = ml_dtypes.bfloat16 if GATHER_DTYPE == "bf16" else np.float32
    xf = np.asarray(x, dtype=np.float32)
    if SEL_STREAM and SEL_FP8:
        xf = xf * meta["dinv"][:, None]
    xg = np.ascontiguousarray(xf.astype(gnp))
    xselfs = None
    if meta["self_static"]:
        cpc = meta["cpc"]
        Q = NODES_PER_CHUNK
        xselfs = []
        for k in range(N_CORES):
            xs = np.zeros((cpc * 2 * 128, xg.shape[1]), gnp)
            for c in range(cpc):
                q = int(meta["chunk_at"][k][c])
                xs[c * 256 : c * 256 + Q] = xg[q * Q : (q + 1) * Q]
            xselfs.append(xs)
    iota = np.tile(np.arange(CHUNK_W, dtype=np.float32), (128, 1))
    w = np.ascontiguousarray(np.asarray(weight, dtype=np.float32))
    bvec = np.zeros((128, 1), np.float32)
    bvec[: len(bias), 0] = np.asarray(bias, dtype=np.float32)
    in_maps = []
    for k in range(N_CORES):
        pc = per_core[k]
        m = dict(xg=xg, idx=pc["idx"], weight=w, bias=bvec)
        if (not SEL_STREAM) or any(meta["B_act"]):
            m.update(
                dstslot=pc["dstslot"],
                normv=pc["normv"],
                ndstslot=-pc["dstslot"],
                nnormv=-pc["normv"],
                iota=iota,
            )
        if SEL_STREAM:
            m["sel"] = pc["sel"]
        if meta["self_static"]:
            m["xself"] = xselfs[k]
        in_maps.append(m)
    return in_maps


def _unshard(results, meta):
    cpc = meta["cpc"]
    outs = [np.asarray(results[k]["out"], dtype=np.float32) for k in range(N_CORES)]
    big = np.concatenate(outs, axis=1)  # [128, ncores*cpc*CHUNK_W]
    chunk_of = meta["chunk_of"]
    rank = meta["rank_of_chunk"][chunk_of]
    core = chunk_of // cpc
    col = core * (cpc * CHUNK_W) + rank * CHUNK_W + meta["slot_of"]
    out = np.ascontiguousarray(big[:, col].T)
    if SEL_STREAM and SEL_FP8:
        # relu(dinv_dst * z) == dinv_dst * relu(z) since dinv_dst > 0
        out *= meta["dinv"][:, None]
    return out


def kernel(x, edge_index, weight, bias):
    global SEL_FP8
    x = np.asarray(x)
    if SEL_FP8 and np.any(np.asarray(bias) != 0):
        SEL_FP8 = False  # fp8 one-hot path relies on bias==0
    per_core, meta = _host_prep(x, edge_index)
    nc = _build_program(meta)
    in_maps = _make_in_maps(x, np.asarray(weight), np.asarray(bias), per_core, meta)
    res = run_bass_kernel_spmd(
        nc,
        in_maps,
        list(range(N_CORES)),
        trace=os.environ.get("GCN_TRACE", "0") == "1",
    )
    LAST_RUN_INFO["exec_time_ns"] = res.exec_time_ns
    LAST_RUN_INFO["meta"] = {
        k: v for k, v in meta.items() if np.isscalar(v)
    } | dict(B_tot=meta["B_tot"])
    return _unshard(res.results, meta)
